# revision 5
# baseline (speedup 1.0000x reference)
"""LSH similarity-matrix kernel for Trainium2 (8 NeuronCores, data-parallel
over batch).

Math: reference computes, per (l, b):
    c1 = (query_embed @ r.T > 0),  c2 = (doc_embed @ r.T > 0)   in {0,1}
    ham = s1 + s2 - 2*c1@c2.T ;  sim = cos(pi/NB * ham), masked where tok==0.
With +-1 codes U = 2c-1 and S = U1 @ U2.T:  ham = (NB - S)/2, so
    sim = sin(pi/(2*NB) * S).
Masks fold into the embeddings: a zeroed embedding row projects to 0,
sign(0) = 0 gives a zero code row, so S = 0 and sin(0) = 0 — exactly the
masked output. Masked doc tokens (half of them: tok in {0,1}) are gathered
away host-side entirely; output columns scatter back as zeros. Batches are
assigned to (core, slot) sorted by active-token count so every core runs an
identically-shaped program with minimal padding per slot.

Precision: the projection runs as a SINGLE float32r (TF32) matmul per chunk
(1 cycle/row at moving >= 256). TF32's 11-bit mantissa flips ~2.8k of the
71M hash bits vs exact fp32; measured end-to-end rel err on the benchmark
data is 6.8e-3 (gate 2e-2). Inputs land in f32r-typed DRAM/SBUF tiles via
plain DMA (f32 bits reinterpret as f32r; the PE rounds internally), so no
engine copies are spent on dtype provenance. The code dot runs as fp8e4m3
DoubleRow matmuls (chunk pairs give K=256 per MM at 2 MACs/cell/cycle);
+-1/0 codes and their fp32 PSUM accumulation are exact. Output is DMA'd as
fp16 (sim in [-1,1]; 5e-4 relative rounding) and cast to f32 host-side.

r is pre-scaled by 2^66 host-side so the DVE sign alternative
clamp(x, -1, 1) = max(min(x,1),-1) is exact (any |proj| > 2^-66 maps to
+-1). Sign work is split between ACT (Sign activation) and DVE (clamp) by
an arrival-aware makespan balancer — GPSIMD/Pool has no PSUM port, so only
these two engines can read matmul results; they are the kernel's
bottleneck (the PE runs at ~60% busy). U2 code layout per slot is
[layer][chunk][pad], making every sign output a contiguous slice and every
code-dot moving operand a simple strided view. The pipeline is
software-skewed per (slot, layer) job: project+sign of job j+1 is emitted
before dot+sin+DMA of job j so the in-order PE queue never parks a dot
behind signs it would stall on; a dummy Sin+Sign at t=0 hoists the 1.3us
activation-table load into the DMA window, and dependency-free warmup
matmuls carry the PE through its p-state clock ramp.
"""
import os
import sys

sys.path.insert(0, "/opt/trn_rl_repo")

from contextlib import ExitStack

import numpy as np

import concourse.bass as bass
import concourse.mybir as mybir
import concourse.tile as tile
from concourse import bacc
from concourse.bass_utils import run_bass_kernel_spmd

L, BAT, A, BDOC, D, NB = 2, 32, 64, 1024, 128, 1024
CORES = 8
BPC = BAT // CORES          # batch slots per core
NJ = BPC * L                # pipeline jobs per core: one per (slot, layer)
CH = NB // 128              # 8 bit-chunks
QPAD = 64                   # query-row cap per (slot, layer) job; the
                            # actual qpad comes from the data (max active
                            # queries, >= 32 so qw = NJ*qpad >= 256)
SCALE = float(2.0 ** 66)
PI = float(np.pi)
N_WARM = 4                  # PE p-state warmup matmuls (512 cols each)

F32 = mybir.dt.float32
F32R = mybir.dt.float32r
F16 = mybir.dt.float16
BF16 = mybir.dt.bfloat16
FP8 = mybir.dt.float8e4
Alu = mybir.AluOpType
Act = mybir.ActivationFunctionType

_BUILD_CACHE: dict = {}

# cost-model constants for the ACT/DVE makespan balancer (ns)
_ACT_NS = 1e9 / 1.2e9
_DVE_NS = 1e9 / 0.96e9
_ACT_INIT = 215.0           # access-latency init + dispatch
_DVE_INIT = 155.0


_BAL_BIAS = [-380.0]
_BAL_OVERRIDE: dict = {}


def _balance(pads_c, jobs, qw):
    """Assign sign ops to ACT ('a') / DVE ('v') with an arrival-aware
    online greedy over the emission sequence: each op becomes available
    when the PE finishes its PSUM unit (a running producer clock), and
    goes to the engine that finishes it first given max(engine-free,
    arrival). ACT additionally absorbs each job's Sin at its stage-C
    position."""
    act = 2 * 198.0 + 1283.0      # dummies + LoadActFuncSet
    dve = 0.0
    pe = 4300.0                   # first projection unit completes ~here
    bias = _BAL_BIAS[0]
    assign = {}

    def put(key, n):
        nonlocal act, dve, pe
        pe += n * 0.4167
        ca = n * _ACT_NS + _ACT_INIT
        cv = n * _DVE_NS + _DVE_INIT
        if max(act, pe) + ca + bias <= max(dve, pe) + cv:
            act = max(act, pe) + ca
            assign[key] = "a"
        else:
            dve = max(dve, pe) + cv
            assign[key] = "v"

    def put_b(j):
        s, _l = jobs[j]
        for k in range(CH):
            put(("d", j, k), pads_c[s])

    put_b(0)
    for h in range(CH // 2):
        put(("q", h), 2 * qw)
    put_b(1)
    for i in range(NJ):
        if i + 2 < NJ:
            put_b(i + 2)
        act += pads_c[jobs[i][0]] * _ACT_NS + _ACT_INIT   # sin(i)
    _balance.totals = (act, dve)
    for k, v in _BAL_OVERRIDE.items():
        if k in assign:
            assign[k] = v
    return assign


def _build(pads_c: tuple, qpad: int = QPAD, reps: int = 1):
    """Per-core SPMD program. pads_c[s]: padded doc width (multiple of 32)
    of batch slot s, shared by both layers. reps > 1 re-emits the whole
    body (timing instrumentation only)."""
    pads_c = tuple(int(p) for p in pads_c)
    pad_cmax = max(pads_c)
    qw = BPC * L * qpad
    # jobs: (slot, layer), slots largest-first so the tail drains the
    # smallest job
    sorder = sorted(range(BPC), key=lambda s: -pads_c[s])
    jobs = [(s, l) for s in sorder for l in range(L)]
    assign = _balance(pads_c, jobs, qw)

    nc = bacc.Bacc("TRN2", target_bir_lowering=False, debug=False)

    QE = nc.dram_tensor("qe", [D, qw], F32R, kind="ExternalInput").ap()
    DE = nc.dram_tensor("de", [BPC, D, 2 * pad_cmax], F32R,
                        kind="ExternalInput").ap()
    RT = nc.dram_tensor("rt", [D, NB], F32R, kind="ExternalInput").ap()
    OUT = nc.dram_tensor("out", [BPC, qpad, 2 * pad_cmax], F16,
                         kind="ExternalOutput").ap()

    with tile.TileContext(nc) as tc, ExitStack() as ctx:
        const = ctx.enter_context(tc.tile_pool(name="const", bufs=1))
        jobp = ctx.enter_context(tc.tile_pool(name="jobp", bufs=3))
        outp = ctx.enter_context(tc.tile_pool(name="outp", bufs=4))
        ps_p = ctx.enter_context(tc.tile_pool(name="ps_p", bufs=4,
                                              space="PSUM"))

        for _rep in range(reps):
            _rp = f"r{_rep}_"
            rt = const.tile([D, NB], F32R, tag="rt", name=f"{_rp}rt")
            qe = const.tile([D, qw], F32R, tag="qe", name=f"{_rp}qe")
            U1 = const.tile([D, CH * qw], FP8, tag="U1", name=f"{_rp}U1")
            warm = const.tile([D, 512], BF16, tag="warm", name=f"{_rp}warm")
            wsin = const.tile([D, 16], F16, tag="wsin", name=f"{_rp}wsin")

            det = {}
            U2 = {}

            def dma_de(j):
                s, l = jobs[j]
                p = pads_c[s]
                if s not in det:
                    det[s] = jobp.tile([D, 2 * pad_cmax], F32R, tag="det",
                                       name=f"{_rp}det{s}")[:, 0:2 * p]
                    U2[s] = jobp.tile([D, L * CH * pad_cmax], FP8,
                                      tag="U2", name=f"{_rp}U2{s}")
                nc.sync.dma_start(out=det[s][:, l * p:(l + 1) * p],
                                  in_=DE[s, :, l * p:(l + 1) * p])

            # ---- startup: chunk-0 weights + the first job's embeddings
            # lead the DMA queue so the first projection starts as early as
            # the fixed DGE/semaphore latency allows ----
            nc.sync.dma_start(out=rt[:, 0:256], in_=RT[:, 0:256])
            dma_de(0)
            nc.sync.dma_start(out=rt[:, 256:NB], in_=RT[:, 256:NB])
            nc.sync.dma_start(out=qe, in_=QE)
            dma_de(1)
            dma_de(2)

            # dummy Sin then Sign on a zeroed tile hoist the single
            # LoadActFuncSet (trig_and_small holds both) into the DMA
            # window; dependency-free dummy matmuls pull the PE through its
            # p-state ramp while the first DMAs land
            nc.gpsimd.memset(warm, 0.0)
            nc.scalar.activation(wsin, warm[:, 0:16], Act.Sin, scale=1.0)
            nc.scalar.activation(wsin, warm[:, 0:16], Act.Sign)
            wps = ps_p.tile([D, 1024], F32, tag="pp", name=f"{_rp}wps")
            for _ in range(N_WARM):
                nc.tensor.matmul(wps[:, 0:512], warm[:, 0:128], warm,
                                 start=True, stop=True)

            def sign_op(key, out_ap, in_ap):
                if assign[key] == "a":
                    nc.scalar.activation(out_ap, in_ap, Act.Sign)
                else:
                    nc.vector.tensor_scalar(out_ap, in_ap, 1.0, -1.0,
                                            Alu.min, Alu.max)

            def stage_b(j):
                """Project job j (one layer of one slot) and sign into its
                slot's U2 range (layout [layer][chunk][p])."""
                s, l = jobs[j]
                p = pads_c[s]
                u2 = U2[s]
                for k in range(CH):
                    ps = ps_p.tile([D, 1024], F32, tag="pp",
                                   name=f"{_rp}pp{j}_{k}")
                    ov = u2[:, (l * CH + k) * p:(l * CH + k + 1) * p]
                    if p <= 512:
                        nc.tensor.matmul(
                            ps[:, 0:p], rt[:, k * 128:(k + 1) * 128],
                            det[s][:, l * p:(l + 1) * p],
                            start=True, stop=True)
                        sign_op(("d", j, k), ov, ps[:, 0:p])
                    else:
                        w = p // 2
                        for jx in range(2):
                            nc.tensor.matmul(
                                ps[:, jx * 512:jx * 512 + w],
                                rt[:, k * 128:(k + 1) * 128],
                                det[s][:, l * p + jx * w:l * p + jx * w + w],
                                start=True, stop=True)
                        iv = ps[:].rearrange("q (a x) -> q a x",
                                             x=512)[:, 0:2, 0:w]
                        sign_op(("d", j, k),
                                ov.rearrange("q (a x) -> q a x", x=w), iv)

            def query_proj():
                for h in range(CH // 2):
                    qp = ps_p.tile([D, 1024], F32, tag="pp",
                                   name=f"{_rp}qp{h}")
                    for i in range(2):
                        k = 2 * h + i
                        nc.tensor.matmul(qp[:, i * 512:i * 512 + qw],
                                         rt[:, k * 128:(k + 1) * 128], qe,
                                         start=True, stop=True)
                    iv = qp[:].rearrange("q (a x) -> q a x",
                                         x=512)[:, 0:2, 0:qw]
                    ov = U1[:, 2 * h * qw:(2 * h + 2) * qw] \
                        .rearrange("q (a x) -> q a x", x=qw)
                    sign_op(("q", h), ov, iv)

            def stage_c(j):
                """Code dot (fp8 DoubleRow), sin, output DMA for job j."""
                s, l = jobs[j]
                p = pads_c[s]
                u2 = U2[s]
                qcol = (s * L + l) * qpad
                pieces = ([(0, p, 0)] if p <= 512
                          else [(0, p // 2, 0), (p // 2, p, 512)])
                S = ps_p.tile([D, 1024], F32, tag="pp", name=f"{_rp}S{j}")
                for c0, c1, p0 in pieces:
                    for jj in range(CH // 2):
                        lw = U1[:, 2 * jj * qw:(2 * jj + 2) * qw] \
                            .rearrange("q (o c) -> q o c", o=2) \
                            [:, :, qcol:qcol + qpad]
                        rv = u2[:, (l * CH + 2 * jj) * p:
                                (l * CH + 2 * jj + 2) * p] \
                            .rearrange("q (o c) -> q o c", o=2) \
                            [:, :, c0:c1]
                        nc.tensor.matmul(
                            S[0:qpad, p0:p0 + c1 - c0], lw, rv,
                            start=(jj == 0), stop=(jj == CH // 2 - 1),
                            perf_mode=mybir.MatmulPerfMode.DoubleRow)
                sim = outp.tile([qpad, pad_cmax], F16, tag="sim",
                                name=f"{_rp}sim{j}")[:, 0:p]
                if p <= 512:
                    nc.scalar.activation(sim, S[0:qpad, 0:p], Act.Sin,
                                         scale=PI / (2.0 * NB))
                else:
                    w = p // 2
                    sv = S[0:qpad, :].rearrange("q (n c) -> q n c",
                                                c=512)[:, 0:2, 0:w]
                    nc.scalar.activation(
                        sim.rearrange("q (n c) -> q n c", c=w), sv,
                        Act.Sin, scale=PI / (2.0 * NB))
                nc.sync.dma_start(out=OUT[s, :, l * p:(l + 1) * p], in_=sim)

            # ---- emission: B(j+1) ahead of C(j); the query projection
            # lands early to feed the engines while doc DMAs trickle in ----
            stage_b(0)
            dma_de(3)
            query_proj()
            stage_b(1)
            for i in range(NJ):
                if i + 4 < NJ:
                    dma_de(i + 4)
                if i + 2 < NJ:
                    stage_b(i + 2)
                stage_c(i)

    nc.compile()
    return nc


def _stage_inputs(query_embed, doc_embed, query_tok, doc_tok, r):
    query_embed = np.ascontiguousarray(query_embed, dtype=np.float32)
    doc_embed = np.ascontiguousarray(doc_embed, dtype=np.float32)
    r = np.ascontiguousarray(r, dtype=np.float32)

    qmask = (np.asarray(query_tok) != 0)
    dmask = (np.asarray(doc_tok) != 0)

    # sort batches by active doc count; slot s takes ranks [s*CORES,
    # (s+1)*CORES) spread across the 8 cores, so per-slot padding is tight
    # and identical on every core (SPMD: one shape per slot)
    counts = dmask.sum(axis=1).astype(int)
    order = np.argsort(counts, kind="stable")
    assign = np.empty((CORES, BPC), dtype=int)   # assign[c, s] = batch id
    for s in range(BPC):
        for c in range(CORES):
            assign[c, s] = order[s * CORES + c]
    pads_c = tuple(
        min(BDOC, max(288, int(-(-int(counts[assign[:, s]].max()) // 8) * 8)))
        for s in range(BPC)
    )
    pad_cmax = max(pads_c)

    qe_m = query_embed * qmask[None, :, :, None].astype(np.float32)
    rts = np.ascontiguousarray(r.T * SCALE)

    # queries compact to their active rows (the mask is per-batch, shared
    # by both layers); qw = NJ*qpad must stay >= 256 for full-rate f32r
    qidxs = [np.flatnonzero(qmask[g]) for g in range(BAT)]
    qpad = min(A, max(32, max(len(q) for q in qidxs)))

    idxs = [np.flatnonzero(dmask[g]) for g in range(BAT)]
    in_maps = []
    for c in range(CORES):
        qe_c = np.zeros((D, NJ * qpad), dtype=np.float32)
        de_c = np.zeros((BPC, D, 2 * pad_cmax), dtype=np.float32)
        for s in range(BPC):
            g = assign[c, s]
            p = pads_c[s]
            idx = idxs[g]
            qi = qidxs[g]
            for li in range(L):
                qe_c[:, (s * L + li) * qpad:(s * L + li) * qpad + len(qi)] \
                    = qe_m[li, g, qi].T
                de_c[s, :, li * p:li * p + len(idx)] = doc_embed[li, g, idx].T
        in_maps.append({"qe": qe_c, "de": de_c, "rt": rts})

    return in_maps, assign, idxs, pads_c, qidxs, qpad


def kernel(query_embed, doc_embed, query_tok, doc_tok, r):
    in_maps, assign, idxs, pads_c, qidxs, qpad = _stage_inputs(
        query_embed, doc_embed, query_tok, doc_tok, r)

    key = (pads_c, qpad)
    if key not in _BUILD_CACHE:
        _BUILD_CACHE[key] = _build(pads_c, qpad)
    nc = _BUILD_CACHE[key]

    res = run_bass_kernel_spmd(nc, in_maps, core_ids=list(range(CORES)))

    out = np.zeros((BAT, L, A, BDOC), dtype=np.float32)
    for c in range(CORES):
        o_c = np.asarray(res.results[c]["out"]).astype(np.float32)
        for s in range(BPC):
            g = assign[c, s]
            p = pads_c[s]
            idx = idxs[g]
            qi = qidxs[g]
            for li in range(L):
                out[g, li][np.ix_(qi, idx)] = \
                    o_c[s, :len(qi), li * p:li * p + len(idx)]
    return out
